# revision 6
# baseline (speedup 1.0000x reference)
"""Trainium2 Bass kernel v2 for nn_Downsample_Spa.

Design:
 - conv emits sigma packed [Mg, 512] per group (group A = block 0, group B =
   blocks 1-3; rows = (b2, hh, o)) via zero-padded block-diagonal lhsT slices;
   the g-pipeline runs once per group. Asymmetric groups shorten the serial
   head: block 0's sigma + g-chain complete while conv B still runs.
 - normalizer folded pre-broadcast: S = ones-matmul over the 9 taps, gbn = gb/S
   in fp16; no [128]-wide reciprocal / final multiply per block.
 - g broadcast to 128 partitions: taps 0-5 via 6 PE one-hot matmuls + 2 ACT
   triple-copies (PSUM fp32 -> SBUF fp16); taps 6-8 via stride-0 DRAM broadcast
   DMA from a 2-replica scratch copy of gbn (replicas dodge HBM bank conflicts).
 - unfold: all-fp16 DVE products (2x mode) + pairwise tree, all on Vector —
   GpSimd tensor ops contend with DVE on SBUF (measured 3-5x slowdown), so
   GpSimd only issues DMAs. Scalar queue issues no DMAs (pure ACT compute).
 - fp16 DMA out (host upconverts); 12 PE warmup matmuls on an early tiny weight
   slice cover the p-state ramp (2.4GHz after ~3us of continuous PE busy).
"""

import os
import sys

import numpy as np

if "/opt/trn_rl_repo" not in sys.path:
    sys.path.insert(0, "/opt/trn_rl_repo")

K = 3
BN_EPS = 1e-5
SIGMA_MIN = 1e-4
N, C, H, W = 8, 64, 128, 128
HO = WO = 64
HH = 2
RS = 65                  # padded-row slots per partition-half
HOC = 32
NBLK = 4
BR = HOC // NBLK         # 8 output rows per block
NPOS = BR * WO           # 512
PL = 3                   # x col-parity planes: w=2j / w=2j+1 / w=2j+2
JW = 66                  # j slots per plane
CR = 17                  # rows per DMA chunk tile (16 + 1 overlap)
GRP = [[0, 1], [2, 3]]   # conv groups (two pipelined g-chains)
MG = [32 * (len(g) - 1) + 18 for g in GRP]   # banded rows (band b2 at 32*b2)
MX = max(MG)             # 114
NWU = 7                  # PE warmup matmuls
REP = 2                  # DRAM replicas of gbn for the tap-6..8 broadcast DMA

_STATE = {}


def _build_consts(conv_w, bn_gamma, bn_beta, bn_mean, bn_var):
    s = (bn_gamma / np.sqrt(bn_var + BN_EPS)).astype(np.float32)
    wf = conv_w.astype(np.float32) * s[:, None, None, None]           # [9,64,3,3]
    bias = (bn_beta - bn_mean * s).astype(np.float32)
    d2 = np.array([(kk // 3 - 1) ** 2 + (kk % 3 - 1) ** 2 for kk in range(9)],
                  np.float32)

    # cs: per-group banded rows concatenated (pads zero)
    # col0 = -0.5*d2[o] (exp scale), col1 = bn_bias - eps
    cs = np.zeros((sum(MG), 2), np.float32)
    off = 0
    for gi, blocks in enumerate(GRP):
        for b2 in range(len(blocks)):
            for hh in range(HH):
                r0 = off + 32 * b2 + hh * 9
                cs[r0:r0 + 9, 0] = -0.5 * d2
                cs[r0:r0 + 9, 1] = bias - SIGMA_MIN
        off += MG[gi]

    # shared conv lhsT [128, tap, 18]: rows (hh, c) -> cols (hh, o)
    wt = np.zeros((128, 9, 18), np.float16)
    for tap in range(9):
        i, j = tap // 3, tap % 3
        for hh in range(HH):
            wt[hh * 64:hh * 64 + 64, tap, hh * 9:hh * 9 + 9] = \
                wf[:, :, i, j].T.astype(np.float16)
    wt = wt.reshape(128, 9 * 18)

    oh1s, ones = [], []
    for gi, blocks in enumerate(GRP):
        nb = len(blocks)
        mg = 32 * (nb - 1) + 18          # banded rows: band b2 at 32*b2
        # broadcast one-hots [mg, (tap0..5, b2), 128]
        oh = np.zeros((mg, 6, nb, 128), np.float16)
        for tap in range(6):
            for b2 in range(nb):
                for hh in range(HH):
                    oh[32 * b2 + hh * 9 + tap, tap, b2, hh * 64:hh * 64 + 64] = 1.0
        oh1s.append(oh.reshape(mg, 6 * nb * 128))

        # S ones [mg, mg]: block-diag 9-tap groups on real rows, identity on pads
        on = np.eye(mg, dtype=np.float16)
        for b2 in range(nb):
            for hh in range(HH):
                r0 = 32 * b2 + hh * 9
                on[r0:r0 + 9, r0:r0 + 9] = 1.0
        ones.append(on)
    return cs, wt, oh1s, ones


def _build_bass(for_sim=False):
    import concourse.bass as bass
    import concourse.tile as tile
    from concourse import mybir

    f32 = mybir.dt.float32
    f16 = mybir.dt.float16
    MULT = mybir.AluOpType.mult
    ADD = mybir.AluOpType.add
    MAX = mybir.AluOpType.max
    AF = mybir.ActivationFunctionType

    if for_sim:
        nc = bass.Bass("TRN2", target_bir_lowering=False, detect_race_conditions=False)
    else:
        from concourse import bacc
        nc = bacc.Bacc()
    xin = nc.dram_tensor("xin", [128, RS, PL, JW], f16, kind="ExternalInput")
    cin = nc.dram_tensor("cin", [sum(MG), 2], f32, kind="ExternalInput")
    win = nc.dram_tensor("win", [128, 9 * 18], f16, kind="ExternalInput")
    gins = [nc.dram_tensor(f"gin{g}", [MG[g], 6 * len(GRP[g]) * 128], f16,
                           kind="ExternalInput") for g in range(len(GRP))]
    oins = [nc.dram_tensor(f"oin{g}", [MG[g], MG[g]], f16, kind="ExternalInput")
            for g in range(len(GRP))]
    gdrs = [nc.dram_tensor(f"gdr{g}", [REP, MG[g], BR, WO], f16, kind="Internal")
            for g in range(len(GRP))]
    out = nc.dram_tensor("out", [128, HOC, WO], f16, kind="ExternalOutput")

    with tile.TileContext(nc) as tc:
        from contextlib import ExitStack
        with ExitStack() as ctx:
            big = ctx.enter_context(tc.tile_pool(name="big", bufs=1))
            gsb = ctx.enter_context(tc.tile_pool(name="gsb", bufs=2))
            gc_p = ctx.enter_context(tc.tile_pool(name="gc", bufs=2))
            yt_p = ctx.enter_context(tc.tile_pool(name="yt", bufs=2))
            tr_p = ctx.enter_context(tc.tile_pool(name="tr", bufs=2))
            ps_a = ctx.enter_context(tc.tile_pool(name="ps_a", bufs=2, space="PSUM"))
            ps_g = ctx.enter_context(tc.tile_pool(name="ps_g", bufs=2, space="PSUM"))

            # --- load order matters. sync + scalar are the HW-DGE queues (fast);
            # the gpsimd queue is software-DGE (slow) -> tiny consts only.
            # scalar queue is free until the first ACT copy (~16us), so it takes
            # the group-A weights (tiny, unblocks warmup) then half the x chunks.
            ws = big.tile([128, 9 * 18], f16)
            with tc.high_priority():
                nc.scalar.dma_start(out=ws[:], in_=win[:])

            xsk = []
            for blk in range(NBLK):
                xs = big.tile([128, CR, PL, JW], f16, tag=f"xs{blk}")
                xsk.append(xs)

            def xdma(eng, blk):
                # whole chunk in one DMA: fully contiguous per partition
                # (4488B single descriptor) for max DMA efficiency
                r0 = 16 * blk
                eng.dma_start(out=xsk[blk][:], in_=xin[:, r0:r0 + CR, :, :])

            xdma(nc.sync, 0)
            xdma(nc.sync, 1)
            xdma(nc.scalar, 2)
            xdma(nc.scalar, 3)
            cstg, osg, gsg = [], [], []
            off = 0
            for g in range(len(GRP)):
                cst = big.tile([MG[g], 2], f32, tag=f"cst{g}")
                nc.gpsimd.dma_start(out=cst[:], in_=cin[off:off + MG[g]])
                cstg.append(cst)
                off += MG[g]
                osn = big.tile([MG[g], MG[g]], f16, tag=f"osn{g}")
                nc.gpsimd.dma_start(out=osn[:], in_=oins[g][:])
                osg.append(osn)
                gst = big.tile([MG[g], 6 * len(GRP[g]) * 128], f16, tag=f"gs{g}")
                nc.gpsimd.dma_start(out=gst[:], in_=gins[g][:])
                gsg.append(gst)
            gdrg = gdrs

            def xtap(tap, blk):
                i, b = tap // 3, tap % 3
                return xsk[blk][:, i:i + 2 * BR - 1:2, b, 0:WO]       # [128, 8, 64]

            def xtaps(i, blk):
                # [128, 3, 8, 64]: (plane, row, col) for row-offset i
                return xsk[blk][:, i:i + 2 * BR - 1:2, 0:PL, 0:WO].transpose([0, 2, 1, 3])

            # ---- PE warm-up on the early tiny weights (p-state ramp) ----
            wu = ps_a.tile([MX, NPOS], f32, tag="ps")
            for _ in range(NWU):
                nc.tensor.matmul(wu[0:18, 0:9 * 18], ws[:, 0:18],
                                 ws[:], start=True, stop=True)


            # ---- conv per group; scheduling floors order the engine queues
            # (in-order queues suffer head-of-line blocking otherwise) ----
            PH_CONV = [[0.001, 0.002], [0.003, 0.004]]
            sigs = []
            for gi, blocks in enumerate(GRP):
                sig = ps_a.tile([MX, NPOS], f32, tag="ps")
                if len(blocks) > 1:
                    # banded layout: define the pad rows between bands so the
                    # full-width g-chain reads finite data (conv matmuls with
                    # start=True then overwrite the real bands)
                    nc.vector.memset(sig[:], 1.0)
                for b2, blk in enumerate(blocks):
                    with tc.tile_wait_until(PH_CONV[gi][b2]):
                        for tap in range(9):
                            # per-b2 accumulation groups (9 matmuls each, banded
                            # at partition 32*b2) keep the PE stream interruptible
                            nc.tensor.matmul(
                                sig[32 * b2:32 * b2 + 18],
                                ws[:, tap * 18:tap * 18 + 18],
                                xtap(tap, blk),
                                start=(tap == 0), stop=(tap == 8),
                                tile_position=(0, 32 * b2),
                            )
                sigs.append(sig)

            def g_emit(gi):
                mg = MG[gi]
                cst = cstg[gi]
                sig = sigs[gi]
                sc = gsb.tile([MX, NPOS], f32, tag="sc")
                nc.vector.tensor_scalar(out=sc[0:mg], in0=sig[0:mg],
                                        scalar1=cst[:, 1:2],
                                        scalar2=float(SIGMA_MIN),
                                        op0=ADD, op1=MAX)
                inv = gsb.tile([MX, NPOS], f32, tag="inv")
                nc.vector.reciprocal_approx_fast(out=inv[0:mg], in_=sc[0:mg])
                qt = gsb.tile([MX, NPOS], f32, tag="qt")
                nc.scalar.activation(out=qt[0:mg], in_=inv[0:mg], func=AF.Square)
                et = gsb.tile([MX, NPOS], f32, tag="et")
                nc.scalar.activation(out=et[0:mg], in_=qt[0:mg], func=AF.Exp,
                                     scale=cst[:, 0:1])
                gb = gsb.tile([MX, NPOS], f16, tag="gb")
                nc.vector.tensor_tensor(out=gb[0:mg], in0=et[0:mg], in1=inv[0:mg],
                                        op=MULT)
                S = ps_a.tile([MX, NPOS], f32, tag="ps")
                nc.tensor.matmul(S[0:mg], osg[gi][:], gb[0:mg], start=True, stop=True)
                rs = gsb.tile([MX, NPOS], f32, tag="rs")
                nc.vector.reciprocal_approx_fast(out=rs[0:mg], in_=S[0:mg])
                gbn = gsb.tile([MX, NPOS], f16, tag="gbn")
                nc.vector.tensor_tensor(out=gbn[0:mg], in0=gb[0:mg], in1=rs[0:mg],
                                        op=MULT)
                # scratch replicas in DRAM for the tap-6..8 broadcast DMA
                for r in range(REP):
                    nc.gpsimd.dma_start(out=gdrg[gi][r], in_=gbn[0:mg])
                return gbn

            def unfold_emit(blk, gbn):
                gi = next(g for g, bl in enumerate(GRP) if blk in bl)
                nb = len(GRP[gi])
                b2 = blk - GRP[gi][0]
                gc = gc_p.tile([128, 9, BR, WO], f16, tag="gc")
                # taps 6-8: stride-0 broadcast DMA from the DRAM replicas
                gdr = gdrg[gi]
                for hh in range(HH):
                    r6 = 32 * b2 + 9 * hh + 6
                    nc.sync.dma_start(
                        out=gc[64 * hh:64 * hh + 32, 6:9],
                        in_=gdr[0, r6:r6 + 3].unsqueeze(0).broadcast_to([32, 3, BR, WO]))
                    nc.scalar.dma_start(
                        out=gc[64 * hh + 32:64 * hh + 64, 6:9],
                        in_=gdr[1, r6:r6 + 3].unsqueeze(0).broadcast_to([32, 3, BR, WO]))
                # taps 0-5: PE one-hot bcast through PSUM + ACT fp16 copy
                mg = MG[gi]
                for tri in range(2):
                    gp = ps_g.tile([128, 3, NPOS], f32, tag="gp")
                    for t in range(3):
                        tap = tri * 3 + t
                        nc.tensor.matmul(
                            gp[:, t],
                            gsg[gi][:, (tap * nb + b2) * 128:(tap * nb + b2 + 1) * 128],
                            gbn[0:mg], start=True, stop=True)
                    nc.scalar.activation(out=gc[:, 3 * tri:3 * tri + 3],
                                         in_=gp[:], func=AF.Copy)

                yt = yt_p.tile([128, 9, BR, WO], f16, tag="yt")
                for i in range(3):
                    nc.vector.tensor_tensor(out=yt[:, 3 * i:3 * i + 3],
                                            in0=xtaps(i, blk),
                                            in1=gc[:, 3 * i:3 * i + 3], op=MULT)

                t4 = tr_p.tile([128, 4, BR, WO], f16, tag="t4")
                nc.vector.tensor_tensor(out=t4[:], in0=yt[:, 0:8:2], in1=yt[:, 1:8:2], op=ADD)
                t2 = tr_p.tile([128, 2, BR, WO], f16, tag="t2")
                nc.vector.tensor_tensor(out=t2[:], in0=t4[:, 0:4:2], in1=t4[:, 1:4:2], op=ADD)
                tA = tr_p.tile([128, BR, WO], f16, tag="tA")
                nc.vector.tensor_tensor(out=tA[:], in0=t2[:, 0], in1=t2[:, 1], op=ADD)
                y = tr_p.tile([128, BR, WO], f16, tag="y")
                nc.vector.tensor_tensor(out=y[:], in0=tA[:], in1=yt[:, 8], op=ADD)
                nc.sync.dma_start(out=out[:, BR * blk:BR * (blk + 1), :], in_=y[:])

            with tc.tile_wait_until(0.0045):
                gbnA = g_emit(0)
            with tc.tile_wait_until(0.005):
                unfold_emit(0, gbnA)
            with tc.tile_wait_until(0.0055):
                gbnB = g_emit(1)
            with tc.tile_wait_until(0.006):
                unfold_emit(1, gbnA)
            with tc.tile_wait_until(0.007):
                unfold_emit(2, gbnB)
            with tc.tile_wait_until(0.008):
                unfold_emit(3, gbnB)

    if not for_sim and not nc.is_finalized():
        nc.finalize()
    return nc


def _prep_inputs(x, conv_w, bn_gamma, bn_beta, bn_mean, bn_var):
    cst, wt, ohs, ones = _build_consts(conv_w, bn_gamma, bn_beta, bn_mean, bn_var)
    xp = np.pad(np.asarray(x, np.float32), ((0, 0), (0, 0), (1, 1), (1, 1)),
                mode="reflect").astype(np.float16)                    # [8,64,130,130]
    in_maps = []
    for n in range(N):
        xc = np.concatenate([xp[n, :, 0:RS, :], xp[n, :, 64:64 + RS, :]], axis=0)
        xpl = np.zeros((128, RS, PL, JW), np.float16)
        xpl[:, :, 0, 0:65] = xc[:, :, 0:130:2]
        xpl[:, :, 1, 0:65] = xc[:, :, 1:130:2]
        xpl[:, :, 2, 0:64] = xc[:, :, 2:130:2]
        im = {"xin": xpl, "cin": cst, "win": wt}
        for g in range(len(GRP)):
            im[f"gin{g}"] = ohs[g]
            im[f"oin{g}"] = ones[g]
        in_maps.append(im)
    return in_maps


def _gather(results):
    out = np.empty((N, C, HO, WO), np.float32)
    for n in range(N):
        d = results[n]["out"].astype(np.float32)
        out[n, :, 0:HOC, :] = d[0:64]
        out[n, :, HOC:, :] = d[64:128]
    return out


def _enable_axon_trace():
    if _STATE.get("trace_hooked"):
        return
    import types
    import antenv
    from concourse import bass_utils
    mod = types.ModuleType("antenv.axon_hooks")
    mod._hook = None
    mod.set_axon_ntff_profile_hook = lambda h: setattr(mod, "_hook", h)
    mod.get_axon_ntff_profile_hook = lambda: mod._hook
    sys.modules["antenv.axon_hooks"] = mod
    antenv.axon_hooks = mod
    from trn_agent_boot.trn_boot import _ntff_profile_via_ctypes
    mod._hook = _ntff_profile_via_ctypes("/opt/axon/libaxon_pjrt.so")
    bass_utils.upload_artifacts = lambda tmpdir: tmpdir
    _STATE["trace_hooked"] = True


def run(x, conv_w, bn_gamma, bn_beta, bn_mean, bn_var, trace=False):
    from concourse.bass_utils import run_bass_kernel_spmd
    if trace:
        _enable_axon_trace()
    if "nc" not in _STATE:
        _STATE["nc"] = _build_bass()
    in_maps = _prep_inputs(x, conv_w, bn_gamma, bn_beta, bn_mean, bn_var)
    res = run_bass_kernel_spmd(_STATE["nc"], in_maps, list(range(N)), trace=trace)
    _STATE["last"] = res
    return _gather(res.results)


def kernel(x, conv_w, bn_gamma, bn_beta, bn_mean, bn_var):
    return run(x, conv_w, bn_gamma, bn_beta, bn_mean, bn_var,
               trace=bool(int(os.environ.get("KERNEL_TRACE", "0"))))


# revision 8
# speedup vs baseline: 1.0385x; 1.0385x over previous
"""Trainium2 Bass kernel v2 for nn_Downsample_Spa.

Design:
 - conv emits sigma packed [Mg, 512] per group (group A = block 0, group B =
   blocks 1-3; rows = (b2, hh, o)) via zero-padded block-diagonal lhsT slices;
   the g-pipeline runs once per group. Asymmetric groups shorten the serial
   head: block 0's sigma + g-chain complete while conv B still runs.
 - normalizer folded pre-broadcast: S = ones-matmul over the 9 taps, gbn = gb/S
   in fp16; no [128]-wide reciprocal / final multiply per block.
 - g broadcast to 128 partitions: taps 0-5 via 6 PE one-hot matmuls + 2 ACT
   triple-copies (PSUM fp32 -> SBUF fp16); taps 6-8 via stride-0 DRAM broadcast
   DMA from a 2-replica scratch copy of gbn (replicas dodge HBM bank conflicts).
 - unfold: all-fp16 DVE products (2x mode) + pairwise tree, all on Vector —
   GpSimd tensor ops contend with DVE on SBUF (measured 3-5x slowdown), so
   GpSimd only issues DMAs. Scalar queue issues no DMAs (pure ACT compute).
 - fp16 DMA out (host upconverts); 12 PE warmup matmuls on an early tiny weight
   slice cover the p-state ramp (2.4GHz after ~3us of continuous PE busy).
"""

import os
import sys

import numpy as np

if "/opt/trn_rl_repo" not in sys.path:
    sys.path.insert(0, "/opt/trn_rl_repo")

K = 3
BN_EPS = 1e-5
SIGMA_MIN = 1e-4
N, C, H, W = 8, 64, 128, 128
HO = WO = 64
HH = 2
RS = 65                  # padded-row slots per partition-half
HOC = 32
NBLK = 4
BR = HOC // NBLK         # 8 output rows per block
NPOS = BR * WO           # 512
PL = 3                   # x col-parity planes: w=2j / w=2j+1 / w=2j+2
JW = 66                  # j slots per plane
CR = 17                  # rows per DMA chunk tile (16 + 1 overlap)
GRP = [[0, 1], [2, 3]]   # conv groups (two pipelined g-chains)
MG = [32 * (len(g) - 1) + 18 for g in GRP]   # banded rows (band b2 at 32*b2)
MX = max(MG)             # 114
NWU = 7                  # PE warmup matmuls
REP = 2                  # DRAM replicas of gbn for the tap-6..8 broadcast DMA

_STATE = {}


def _build_consts(conv_w, bn_gamma, bn_beta, bn_mean, bn_var):
    s = (bn_gamma / np.sqrt(bn_var + BN_EPS)).astype(np.float32)
    wf = conv_w.astype(np.float32) * s[:, None, None, None]           # [9,64,3,3]
    bias = (bn_beta - bn_mean * s).astype(np.float32)
    d2 = np.array([(kk // 3 - 1) ** 2 + (kk % 3 - 1) ** 2 for kk in range(9)],
                  np.float32)

    # cs: per-group banded rows concatenated (pads zero)
    # col0 = -0.5*d2[o] (exp scale), col1 = bn_bias - eps
    cs = np.zeros((sum(MG), 2), np.float32)
    off = 0
    for gi, blocks in enumerate(GRP):
        for b2 in range(len(blocks)):
            for hh in range(HH):
                r0 = off + 32 * b2 + hh * 9
                cs[r0:r0 + 9, 0] = -0.5 * d2
                cs[r0:r0 + 9, 1] = bias - SIGMA_MIN
        off += MG[gi]

    # shared conv lhsT [128, tap, 18]: rows (hh, c) -> cols (hh, o)
    wt = np.zeros((128, 9, 18), np.float16)
    for tap in range(9):
        i, j = tap // 3, tap % 3
        for hh in range(HH):
            wt[hh * 64:hh * 64 + 64, tap, hh * 9:hh * 9 + 9] = \
                wf[:, :, i, j].T.astype(np.float16)
    wt = wt.reshape(128, 9 * 18)

    oh1s, ones = [], []
    for gi, blocks in enumerate(GRP):
        nb = len(blocks)
        mg = 32 * (nb - 1) + 18          # banded rows: band b2 at 32*b2
        # broadcast one-hots [mg, (tap0..5, b2), 128]
        oh = np.zeros((mg, 6, nb, 128), np.float16)
        for tap in range(6):
            for b2 in range(nb):
                for hh in range(HH):
                    oh[32 * b2 + hh * 9 + tap, tap, b2, hh * 64:hh * 64 + 64] = 1.0
        oh1s.append(oh.reshape(mg, 6 * nb * 128))

        # S ones [mg, mg]: block-diag 9-tap groups on real rows, identity on pads
        on = np.eye(mg, dtype=np.float16)
        for b2 in range(nb):
            for hh in range(HH):
                r0 = 32 * b2 + hh * 9
                on[r0:r0 + 9, r0:r0 + 9] = 1.0
        ones.append(on)
    return cs, wt, oh1s, ones


def _build_bass(for_sim=False):
    import concourse.bass as bass
    import concourse.tile as tile
    from concourse import mybir

    f32 = mybir.dt.float32
    f16 = mybir.dt.float16
    MULT = mybir.AluOpType.mult
    ADD = mybir.AluOpType.add
    MAX = mybir.AluOpType.max
    AF = mybir.ActivationFunctionType

    if for_sim:
        nc = bass.Bass("TRN2", target_bir_lowering=False, detect_race_conditions=False)
    else:
        from concourse import bacc
        nc = bacc.Bacc()
    xin = nc.dram_tensor("xin", [128, RS, 2, JW], f16, kind="ExternalInput")
    cin = nc.dram_tensor("cin", [sum(MG), 2], f32, kind="ExternalInput")
    win = nc.dram_tensor("win", [128, 9 * 18], f16, kind="ExternalInput")
    gins = [nc.dram_tensor(f"gin{g}", [MG[g], 6 * len(GRP[g]) * 128], f16,
                           kind="ExternalInput") for g in range(len(GRP))]
    oins = [nc.dram_tensor(f"oin{g}", [MG[g], MG[g]], f16, kind="ExternalInput")
            for g in range(len(GRP))]
    gdrs = [nc.dram_tensor(f"gdr{g}", [REP, MG[g], BR, WO], f16, kind="Internal")
            for g in range(len(GRP))]
    out = nc.dram_tensor("out", [128, HOC, WO], f16, kind="ExternalOutput")

    with tile.TileContext(nc) as tc:
        from contextlib import ExitStack
        with ExitStack() as ctx:
            big = ctx.enter_context(tc.tile_pool(name="big", bufs=1))
            gsb = ctx.enter_context(tc.tile_pool(name="gsb", bufs=2))
            gc_p = ctx.enter_context(tc.tile_pool(name="gc", bufs=2))
            yt_p = ctx.enter_context(tc.tile_pool(name="yt", bufs=2))
            tr_p = ctx.enter_context(tc.tile_pool(name="tr", bufs=2))
            ps_a = ctx.enter_context(tc.tile_pool(name="ps_a", bufs=2, space="PSUM"))
            ps_g = ctx.enter_context(tc.tile_pool(name="ps_g", bufs=2, space="PSUM"))

            # --- load order matters. sync + scalar are the HW-DGE queues (fast);
            # the gpsimd queue is software-DGE (slow) -> tiny consts only.
            # scalar queue is free until the first ACT copy (~16us), so it takes
            # the group-A weights (tiny, unblocks warmup) then half the x chunks.
            ws = big.tile([128, 9 * 18], f16)
            with tc.high_priority():
                nc.scalar.dma_start(out=ws[:], in_=win[:])

            xsk = []
            for blk in range(NBLK):
                xs = big.tile([128, CR, 2, JW], f16, tag=f"xs{blk}")
                xsk.append(xs)

            def xdma(eng, blk):
                # whole chunk in one DMA: fully contiguous per partition
                # (4488B single descriptor) for max DMA efficiency
                r0 = 16 * blk
                eng.dma_start(out=xsk[blk][:], in_=xin[:, r0:r0 + CR, :, :])

            xdma(nc.sync, 0)
            xdma(nc.sync, 1)
            xdma(nc.scalar, 2)
            xdma(nc.scalar, 3)
            cstg, osg, gsg = [], [], []
            off = 0
            for g in range(len(GRP)):
                cst = big.tile([MG[g], 2], f32, tag=f"cst{g}")
                nc.gpsimd.dma_start(out=cst[:], in_=cin[off:off + MG[g]])
                cstg.append(cst)
                off += MG[g]
                osn = big.tile([MG[g], MG[g]], f16, tag=f"osn{g}")
                nc.gpsimd.dma_start(out=osn[:], in_=oins[g][:])
                osg.append(osn)
                gst = big.tile([MG[g], 6 * len(GRP[g]) * 128], f16, tag=f"gs{g}")
                nc.gpsimd.dma_start(out=gst[:], in_=gins[g][:])
                gsg.append(gst)
            gdrg = gdrs

            def xtap(tap, blk):
                i, b = tap // 3, tap % 3
                if b == 2:
                    # plane 2 content == plane 0 shifted one j-slot; read the
                    # shifted view (only planes 0-1 are stored)
                    return xsk[blk][:, i:i + 2 * BR - 1:2, 0, 1:WO + 1]
                return xsk[blk][:, i:i + 2 * BR - 1:2, b, 0:WO]       # [128, 8, 64]

            def xtaps2(i, blk):
                # [128, 2, 8, 64]: (plane, row, col) for row-offset i, planes 0-1
                return xsk[blk][:, i:i + 2 * BR - 1:2, 0:2, 0:WO].transpose([0, 2, 1, 3])

            # ---- PE warm-up on the early tiny weights (p-state ramp) ----
            wu = ps_a.tile([MX, NPOS], f32, tag="ps")
            for _ in range(NWU):
                nc.tensor.matmul(wu[0:18, 0:9 * 18], ws[:, 0:18],
                                 ws[:], start=True, stop=True)


            # ---- conv per group; scheduling floors order the engine queues
            # (in-order queues suffer head-of-line blocking otherwise) ----
            PH_CONV = [[0.001, 0.002], [0.003, 0.004]]
            sigs = []
            for gi, blocks in enumerate(GRP):
                sig = ps_a.tile([MX, NPOS], f32, tag="ps")
                if len(blocks) > 1:
                    # banded layout: define the pad rows between bands so the
                    # full-width g-chain reads finite data (conv matmuls with
                    # start=True then overwrite the real bands)
                    nc.vector.memset(sig[:], 1.0)
                for b2, blk in enumerate(blocks):
                    with tc.tile_wait_until(PH_CONV[gi][b2]):
                        for tap in range(9):
                            # per-b2 accumulation groups (9 matmuls each, banded
                            # at partition 32*b2) keep the PE stream interruptible
                            nc.tensor.matmul(
                                sig[32 * b2:32 * b2 + 18],
                                ws[:, tap * 18:tap * 18 + 18],
                                xtap(tap, blk),
                                start=(tap == 0), stop=(tap == 8),
                                tile_position=(0, 32 * b2),
                            )
                sigs.append(sig)

            def g_emit(gi):
                mg = MG[gi]
                cst = cstg[gi]
                sig = sigs[gi]
                sc = gsb.tile([MX, NPOS], f32, tag="sc")
                nc.vector.tensor_scalar(out=sc[0:mg], in0=sig[0:mg],
                                        scalar1=cst[:, 1:2],
                                        scalar2=float(SIGMA_MIN),
                                        op0=ADD, op1=MAX)
                inv = gsb.tile([MX, NPOS], f32, tag="inv")
                nc.vector.reciprocal_approx_fast(out=inv[0:mg], in_=sc[0:mg])
                qt = gsb.tile([MX, NPOS], f32, tag="qt")
                nc.scalar.activation(out=qt[0:mg], in_=inv[0:mg], func=AF.Square)
                et = gsb.tile([MX, NPOS], f32, tag="et")
                nc.scalar.activation(out=et[0:mg], in_=qt[0:mg], func=AF.Exp,
                                     scale=cst[:, 0:1])
                gb = gsb.tile([MX, NPOS], f16, tag="gb")
                nc.vector.tensor_tensor(out=gb[0:mg], in0=et[0:mg], in1=inv[0:mg],
                                        op=MULT)
                S = ps_a.tile([MX, NPOS], f32, tag="ps")
                nc.tensor.matmul(S[0:mg], osg[gi][:], gb[0:mg], start=True, stop=True)
                rs = gsb.tile([MX, NPOS], f32, tag="rs")
                nc.vector.reciprocal_approx_fast(out=rs[0:mg], in_=S[0:mg])
                gbn = gsb.tile([MX, NPOS], f16, tag="gbn")
                nc.vector.tensor_tensor(out=gbn[0:mg], in0=gb[0:mg], in1=rs[0:mg],
                                        op=MULT)
                # scratch replicas in DRAM for the tap-6..8 broadcast DMA
                for r in range(REP):
                    nc.gpsimd.dma_start(out=gdrg[gi][r], in_=gbn[0:mg])
                return gbn

            def unfold_emit(blk, gbn):
                gi = next(g for g, bl in enumerate(GRP) if blk in bl)
                nb = len(GRP[gi])
                b2 = blk - GRP[gi][0]
                gc = gc_p.tile([128, 9, BR, WO], f16, tag="gc")
                # taps 6-8: stride-0 broadcast DMA from the DRAM replicas
                gdr = gdrg[gi]
                for hh in range(HH):
                    r6 = 32 * b2 + 9 * hh + 6
                    nc.sync.dma_start(
                        out=gc[64 * hh:64 * hh + 32, 6:9],
                        in_=gdr[0, r6:r6 + 3].unsqueeze(0).broadcast_to([32, 3, BR, WO]))
                    nc.scalar.dma_start(
                        out=gc[64 * hh + 32:64 * hh + 64, 6:9],
                        in_=gdr[1, r6:r6 + 3].unsqueeze(0).broadcast_to([32, 3, BR, WO]))
                # taps 0-5: PE one-hot bcast through PSUM + ACT fp16 copy
                mg = MG[gi]
                for tri in range(2):
                    gp = ps_g.tile([128, 3, NPOS], f32, tag="gp")
                    for t in range(3):
                        tap = tri * 3 + t
                        nc.tensor.matmul(
                            gp[:, t],
                            gsg[gi][:, (tap * nb + b2) * 128:(tap * nb + b2 + 1) * 128],
                            gbn[0:mg], start=True, stop=True)
                    nc.scalar.activation(out=gc[:, 3 * tri:3 * tri + 3],
                                         in_=gp[:], func=AF.Copy)

                yt = yt_p.tile([128, 9, BR, WO], f16, tag="yt")
                # all 6 plane-0/1 taps in ONE op and the 3 shifted singles in a
                # second: hand-built APs decompose the x row index as i + 2r
                xb, gcb, ytb = xsk[blk][:], gc[:], yt[:]
                XP, GP = CR * 2 * JW, 9 * BR * WO
                x6 = bass.AP(xb.tensor, xb.offset,
                             [[XP, 128], [2 * JW, 3], [JW, 2], [4 * JW, BR], [1, WO]])
                g6 = bass.AP(gcb.tensor, gcb.offset,
                             [[GP, 128], [3 * NPOS, 3], [NPOS, 2], [WO, BR], [1, WO]])
                y6 = bass.AP(ytb.tensor, ytb.offset,
                             [[GP, 128], [3 * NPOS, 3], [NPOS, 2], [WO, BR], [1, WO]])
                nc.vector.tensor_tensor(out=y6, in0=x6, in1=g6, op=MULT)
                x3 = bass.AP(xb.tensor, xb.offset + 1,
                             [[XP, 128], [2 * JW, 3], [4 * JW, BR], [1, WO]])
                g3 = bass.AP(gcb.tensor, gcb.offset + 2 * NPOS,
                             [[GP, 128], [3 * NPOS, 3], [WO, BR], [1, WO]])
                y3 = bass.AP(ytb.tensor, ytb.offset + 2 * NPOS,
                             [[GP, 128], [3 * NPOS, 3], [WO, BR], [1, WO]])
                nc.vector.tensor_tensor(out=y3, in0=x3, in1=g3, op=MULT)

                t4 = tr_p.tile([128, 4, BR, WO], f16, tag="t4")
                nc.vector.tensor_tensor(out=t4[:], in0=yt[:, 0:8:2], in1=yt[:, 1:8:2], op=ADD)
                t2 = tr_p.tile([128, 2, BR, WO], f16, tag="t2")
                nc.vector.tensor_tensor(out=t2[:], in0=t4[:, 0:4:2], in1=t4[:, 1:4:2], op=ADD)
                tA = tr_p.tile([128, BR, WO], f16, tag="tA")
                nc.vector.tensor_tensor(out=tA[:], in0=t2[:, 0], in1=t2[:, 1], op=ADD)
                y = tr_p.tile([128, BR, WO], f16, tag="y")
                nc.vector.tensor_tensor(out=y[:], in0=tA[:], in1=yt[:, 8], op=ADD)
                nc.sync.dma_start(out=out[:, BR * blk:BR * (blk + 1), :], in_=y[:])

            with tc.tile_wait_until(0.0045):
                gbnA = g_emit(0)
            with tc.tile_wait_until(0.005):
                unfold_emit(0, gbnA)
            with tc.tile_wait_until(0.0055):
                gbnB = g_emit(1)
            with tc.tile_wait_until(0.006):
                unfold_emit(1, gbnA)
            with tc.tile_wait_until(0.007):
                unfold_emit(2, gbnB)
            with tc.tile_wait_until(0.008):
                unfold_emit(3, gbnB)

    if not for_sim and not nc.is_finalized():
        nc.finalize()
    return nc


def _prep_inputs(x, conv_w, bn_gamma, bn_beta, bn_mean, bn_var):
    cst, wt, ohs, ones = _build_consts(conv_w, bn_gamma, bn_beta, bn_mean, bn_var)
    xp = np.pad(np.asarray(x, np.float32), ((0, 0), (0, 0), (1, 1), (1, 1)),
                mode="reflect").astype(np.float16)                    # [8,64,130,130]
    in_maps = []
    for n in range(N):
        xc = np.concatenate([xp[n, :, 0:RS, :], xp[n, :, 64:64 + RS, :]], axis=0)
        xpl = np.zeros((128, RS, 2, JW), np.float16)
        xpl[:, :, 0, 0:65] = xc[:, :, 0:130:2]
        xpl[:, :, 1, 0:65] = xc[:, :, 1:130:2]
        im = {"xin": xpl, "cin": cst, "win": wt}
        for g in range(len(GRP)):
            im[f"gin{g}"] = ohs[g]
            im[f"oin{g}"] = ones[g]
        in_maps.append(im)
    return in_maps


def _gather(results):
    out = np.empty((N, C, HO, WO), np.float32)
    for n in range(N):
        d = results[n]["out"].astype(np.float32)
        out[n, :, 0:HOC, :] = d[0:64]
        out[n, :, HOC:, :] = d[64:128]
    return out


def _enable_axon_trace():
    if _STATE.get("trace_hooked"):
        return
    import types
    import antenv
    from concourse import bass_utils
    mod = types.ModuleType("antenv.axon_hooks")
    mod._hook = None
    mod.set_axon_ntff_profile_hook = lambda h: setattr(mod, "_hook", h)
    mod.get_axon_ntff_profile_hook = lambda: mod._hook
    sys.modules["antenv.axon_hooks"] = mod
    antenv.axon_hooks = mod
    from trn_agent_boot.trn_boot import _ntff_profile_via_ctypes
    mod._hook = _ntff_profile_via_ctypes("/opt/axon/libaxon_pjrt.so")
    bass_utils.upload_artifacts = lambda tmpdir: tmpdir
    _STATE["trace_hooked"] = True


def run(x, conv_w, bn_gamma, bn_beta, bn_mean, bn_var, trace=False):
    from concourse.bass_utils import run_bass_kernel_spmd
    if trace:
        _enable_axon_trace()
    if "nc" not in _STATE:
        _STATE["nc"] = _build_bass()
    in_maps = _prep_inputs(x, conv_w, bn_gamma, bn_beta, bn_mean, bn_var)
    res = run_bass_kernel_spmd(_STATE["nc"], in_maps, list(range(N)), trace=trace)
    _STATE["last"] = res
    return _gather(res.results)


def kernel(x, conv_w, bn_gamma, bn_beta, bn_mean, bn_var):
    return run(x, conv_w, bn_gamma, bn_beta, bn_mean, bn_var,
               trace=bool(int(os.environ.get("KERNEL_TRACE", "0"))))


# revision 9
# speedup vs baseline: 1.1056x; 1.0646x over previous
"""Trainium2 Bass kernel v2 for nn_Downsample_Spa.

Design:
 - conv emits sigma packed [Mg, 512] per group (group A = block 0, group B =
   blocks 1-3; rows = (b2, hh, o)) via zero-padded block-diagonal lhsT slices;
   the g-pipeline runs once per group. Asymmetric groups shorten the serial
   head: block 0's sigma + g-chain complete while conv B still runs.
 - normalizer folded pre-broadcast: S = ones-matmul over the 9 taps, gbn = gb/S
   in fp16; no [128]-wide reciprocal / final multiply per block.
 - g broadcast to 128 partitions: taps 0-5 via 6 PE one-hot matmuls + 2 ACT
   triple-copies (PSUM fp32 -> SBUF fp16); taps 6-8 via stride-0 DRAM broadcast
   DMA from a 2-replica scratch copy of gbn (replicas dodge HBM bank conflicts).
 - unfold: all-fp16 DVE products (2x mode) + pairwise tree, all on Vector —
   GpSimd tensor ops contend with DVE on SBUF (measured 3-5x slowdown), so
   GpSimd only issues DMAs. Scalar queue issues no DMAs (pure ACT compute).
 - fp16 DMA out (host upconverts); 12 PE warmup matmuls on an early tiny weight
   slice cover the p-state ramp (2.4GHz after ~3us of continuous PE busy).
"""

import os
import sys

import numpy as np

if "/opt/trn_rl_repo" not in sys.path:
    sys.path.insert(0, "/opt/trn_rl_repo")

K = 3
BN_EPS = 1e-5
SIGMA_MIN = 1e-4
N, C, H, W = 8, 64, 128, 128
HO = WO = 64
HH = 2
RS = 65                  # padded-row slots per partition-half
HOC = 32
NBLK = 4
BR = HOC // NBLK         # 8 output rows per block
NPOS = BR * WO           # 512
PL = 3                   # x col-parity planes: w=2j / w=2j+1 / w=2j+2
JW = 66                  # j slots per plane
CR = 17                  # rows per DMA chunk tile (16 + 1 overlap)
GRP = [[0, 1], [2, 3]]   # conv groups (two pipelined g-chains)
MG = [32 * (len(g) - 1) + 18 for g in GRP]   # banded rows (band b2 at 32*b2)
MX = max(MG)             # 114
NWU = 7                  # PE warmup matmuls
REP = 2                  # DRAM replicas of gbn for the tap-6..8 broadcast DMA

_STATE = {}


def _build_consts(conv_w, bn_gamma, bn_beta, bn_mean, bn_var):
    s = (bn_gamma / np.sqrt(bn_var + BN_EPS)).astype(np.float32)
    wf = conv_w.astype(np.float32) * s[:, None, None, None]           # [9,64,3,3]
    bias = (bn_beta - bn_mean * s).astype(np.float32)
    d2 = np.array([(kk // 3 - 1) ** 2 + (kk % 3 - 1) ** 2 for kk in range(9)],
                  np.float32)

    # cs: per-group banded rows concatenated (pads zero)
    # col0 = -0.5*d2[o] (exp scale), col1 = bn_bias - eps
    cs = np.zeros((sum(MG), 2), np.float32)
    off = 0
    for gi, blocks in enumerate(GRP):
        for b2 in range(len(blocks)):
            for hh in range(HH):
                r0 = off + 32 * b2 + hh * 9
                cs[r0:r0 + 9, 0] = -0.5 * d2
                cs[r0:r0 + 9, 1] = bias - SIGMA_MIN
        off += MG[gi]

    # shared conv lhsT [128, tap, 18]: rows (hh, c) -> cols (hh, o)
    wt = np.zeros((128, 9, 18), np.float16)
    for tap in range(9):
        i, j = tap // 3, tap % 3
        for hh in range(HH):
            wt[hh * 64:hh * 64 + 64, tap, hh * 9:hh * 9 + 9] = \
                wf[:, :, i, j].T.astype(np.float16)
    wt = wt.reshape(128, 9 * 18)

    oh1s, ones = [], []
    for gi, blocks in enumerate(GRP):
        nb = len(blocks)
        mg = 32 * (nb - 1) + 18          # banded rows: band b2 at 32*b2
        # broadcast one-hots [mg, (tap0..5, b2), 128]
        oh = np.zeros((mg, 6, nb, 128), np.float16)
        for tap in range(6):
            for b2 in range(nb):
                for hh in range(HH):
                    oh[32 * b2 + hh * 9 + tap, tap, b2, hh * 64:hh * 64 + 64] = 1.0
        oh1s.append(oh.reshape(mg, 6 * nb * 128))

        # S ones [mg, mg]: block-diag 9-tap groups on real rows, identity on pads
        on = np.eye(mg, dtype=np.float16)
        for b2 in range(nb):
            for hh in range(HH):
                r0 = 32 * b2 + hh * 9
                on[r0:r0 + 9, r0:r0 + 9] = 1.0
        ones.append(on)
    return cs, wt, oh1s, ones


def _build_bass(for_sim=False):
    import concourse.bass as bass
    import concourse.tile as tile
    from concourse import mybir

    f32 = mybir.dt.float32
    f16 = mybir.dt.float16
    MULT = mybir.AluOpType.mult
    ADD = mybir.AluOpType.add
    MAX = mybir.AluOpType.max
    AF = mybir.ActivationFunctionType

    if for_sim:
        nc = bass.Bass("TRN2", target_bir_lowering=False, detect_race_conditions=False)
    else:
        from concourse import bacc
        nc = bacc.Bacc()
    xin = nc.dram_tensor("xin", [128, RS, 2, JW], f16, kind="ExternalInput")
    cin = nc.dram_tensor("cin", [sum(MG), 2], f32, kind="ExternalInput")
    win = nc.dram_tensor("win", [128, 9 * 18], f16, kind="ExternalInput")
    gins = [nc.dram_tensor(f"gin{g}", [MG[g], 6 * len(GRP[g]) * 128], f16,
                           kind="ExternalInput") for g in range(len(GRP))]
    oins = [nc.dram_tensor(f"oin{g}", [MG[g], MG[g]], f16, kind="ExternalInput")
            for g in range(len(GRP))]
    gdrs = [nc.dram_tensor(f"gdr{g}", [REP, MG[g], BR, WO], f16, kind="Internal")
            for g in range(len(GRP))]
    out = nc.dram_tensor("out", [128, HOC, WO], f16, kind="ExternalOutput")

    with tile.TileContext(nc) as tc:
        from contextlib import ExitStack
        with ExitStack() as ctx:
            big = ctx.enter_context(tc.tile_pool(name="big", bufs=1))
            gsb = ctx.enter_context(tc.tile_pool(name="gsb", bufs=2))
            gc_p = ctx.enter_context(tc.tile_pool(name="gc", bufs=2))
            yt_p = ctx.enter_context(tc.tile_pool(name="yt", bufs=2))
            tr_p = ctx.enter_context(tc.tile_pool(name="tr", bufs=2))
            ps_a = ctx.enter_context(tc.tile_pool(name="ps_a", bufs=2, space="PSUM"))
            ps_g = ctx.enter_context(tc.tile_pool(name="ps_g", bufs=2, space="PSUM"))

            # --- load order matters. sync + scalar are the HW-DGE queues (fast);
            # the gpsimd queue is software-DGE (slow) -> tiny consts only.
            # scalar queue is free until the first ACT copy (~16us), so it takes
            # the group-A weights (tiny, unblocks warmup) then half the x chunks.
            ws = big.tile([128, 9 * 18], f16)
            with tc.high_priority():
                nc.scalar.dma_start(out=ws[:], in_=win[:])

            xsk = []
            for blk in range(NBLK):
                xs = big.tile([128, CR, 2, JW], f16, tag=f"xs{blk}")
                xsk.append(xs)

            def xdma(eng, blk):
                # whole chunk in one DMA: fully contiguous per partition
                # (4488B single descriptor) for max DMA efficiency
                r0 = 16 * blk
                eng.dma_start(out=xsk[blk][:], in_=xin[:, r0:r0 + CR, :, :])

            xdma(nc.sync, 0)
            xdma(nc.sync, 1)
            xdma(nc.scalar, 2)
            xdma(nc.scalar, 3)
            cstg, osg, gsg = [], [], []
            off = 0
            for g in range(len(GRP)):
                cst = big.tile([MG[g], 2], f32, tag=f"cst{g}")
                nc.gpsimd.dma_start(out=cst[:], in_=cin[off:off + MG[g]])
                cstg.append(cst)
                off += MG[g]
                osn = big.tile([MG[g], MG[g]], f16, tag=f"osn{g}")
                nc.gpsimd.dma_start(out=osn[:], in_=oins[g][:])
                osg.append(osn)
                gst = big.tile([MG[g], 6 * len(GRP[g]) * 128], f16, tag=f"gs{g}")
                nc.gpsimd.dma_start(out=gst[:], in_=gins[g][:])
                gsg.append(gst)
            gdrg = gdrs

            def xtap(tap, blk):
                i, b = tap // 3, tap % 3
                if b == 2:
                    # plane 2 content == plane 0 shifted one j-slot; read the
                    # shifted view (only planes 0-1 are stored)
                    return xsk[blk][:, i:i + 2 * BR - 1:2, 0, 1:WO + 1]
                return xsk[blk][:, i:i + 2 * BR - 1:2, b, 0:WO]       # [128, 8, 64]

            def xtaps2(i, blk):
                # [128, 2, 8, 64]: (plane, row, col) for row-offset i, planes 0-1
                return xsk[blk][:, i:i + 2 * BR - 1:2, 0:2, 0:WO].transpose([0, 2, 1, 3])

            # ---- PE warm-up on the early tiny weights (p-state ramp) ----
            wu = ps_a.tile([MX, NPOS], f32, tag="ps")
            for _ in range(NWU):
                nc.tensor.matmul(wu[0:18, 0:9 * 18], ws[:, 0:18],
                                 ws[:], start=True, stop=True)


            # ---- conv per group; scheduling floors order the engine queues
            # (in-order queues suffer head-of-line blocking otherwise) ----
            PH_CONV = [[0.001, 0.002], [0.003, 0.004]]
            sigs = []
            for gi, blocks in enumerate(GRP):
                sig = ps_a.tile([MX, NPOS], f32, tag="ps")
                if len(blocks) > 1:
                    # banded layout: define the pad rows between bands so the
                    # full-width g-chain reads finite data (conv matmuls with
                    # start=True then overwrite the real bands)
                    nc.vector.memset(sig[:], 1.0)
                for b2, blk in enumerate(blocks):
                    with tc.tile_wait_until(PH_CONV[gi][b2]):
                        for tap in range(9):
                            # per-b2 accumulation groups (9 matmuls each, banded
                            # at partition 32*b2) keep the PE stream interruptible
                            nc.tensor.matmul(
                                sig[32 * b2:32 * b2 + 18],
                                ws[:, tap * 18:tap * 18 + 18],
                                xtap(tap, blk),
                                start=(tap == 0), stop=(tap == 8),
                                tile_position=(0, 32 * b2),
                            )
                sigs.append(sig)

            def g_emit(gi):
                mg = MG[gi]
                cst = cstg[gi]
                sig = sigs[gi]
                sc = gsb.tile([MX, NPOS], f32, tag="sc")
                nc.vector.tensor_scalar(out=sc[0:mg], in0=sig[0:mg],
                                        scalar1=cst[:, 1:2],
                                        scalar2=float(SIGMA_MIN),
                                        op0=ADD, op1=MAX)
                inv = gsb.tile([MX, NPOS], f32, tag="inv")
                nc.vector.reciprocal_approx_fast(out=inv[0:mg], in_=sc[0:mg])
                qt = gsb.tile([MX, NPOS], f32, tag="qt")
                nc.scalar.activation(out=qt[0:mg], in_=inv[0:mg], func=AF.Square)
                et = gsb.tile([MX, NPOS], f32, tag="et")
                nc.scalar.activation(out=et[0:mg], in_=qt[0:mg], func=AF.Exp,
                                     scale=cst[:, 0:1])
                gb = gsb.tile([MX, NPOS], f16, tag="gb")
                nc.vector.tensor_tensor(out=gb[0:mg], in0=et[0:mg], in1=inv[0:mg],
                                        op=MULT)
                S = ps_a.tile([MX, NPOS], f32, tag="ps")
                nc.tensor.matmul(S[0:mg], osg[gi][:], gb[0:mg], start=True, stop=True)
                rs = gsb.tile([MX, NPOS], f32, tag="rs")
                nc.vector.reciprocal_approx_fast(out=rs[0:mg], in_=S[0:mg])
                gbn = gsb.tile([MX, NPOS], f16, tag="gbn")
                nc.vector.tensor_tensor(out=gbn[0:mg], in0=gb[0:mg], in1=rs[0:mg],
                                        op=MULT)
                # scratch replicas in DRAM for the tap-6..8 broadcast DMA
                # (HW-DGE queues: the gpsimd SW queue adds ~2us to this hop)
                nc.sync.dma_start(out=gdrg[gi][0], in_=gbn[0:mg])
                nc.scalar.dma_start(out=gdrg[gi][1], in_=gbn[0:mg])
                return gbn

            def unfold_emit(blk, gbn):
                gi = next(g for g, bl in enumerate(GRP) if blk in bl)
                nb = len(GRP[gi])
                b2 = blk - GRP[gi][0]
                gc = gc_p.tile([128, 9, BR, WO], f16, tag="gc")
                # taps 6-8: stride-0 broadcast DMA from the DRAM replicas
                gdr = gdrg[gi]
                for hh in range(HH):
                    r6 = 32 * b2 + 9 * hh + 6
                    nc.sync.dma_start(
                        out=gc[64 * hh:64 * hh + 32, 6:9],
                        in_=gdr[0, r6:r6 + 3].unsqueeze(0).broadcast_to([32, 3, BR, WO]))
                    nc.scalar.dma_start(
                        out=gc[64 * hh + 32:64 * hh + 64, 6:9],
                        in_=gdr[1, r6:r6 + 3].unsqueeze(0).broadcast_to([32, 3, BR, WO]))
                # taps 0-5: PE one-hot bcast through PSUM + ACT fp16 copy
                mg = MG[gi]
                for tri in range(2):
                    gp = ps_g.tile([128, 3, NPOS], f32, tag="gp")
                    for t in range(3):
                        tap = tri * 3 + t
                        nc.tensor.matmul(
                            gp[:, t],
                            gsg[gi][:, (tap * nb + b2) * 128:(tap * nb + b2 + 1) * 128],
                            gbn[0:mg], start=True, stop=True)
                    nc.scalar.activation(out=gc[:, 3 * tri:3 * tri + 3],
                                         in_=gp[:], func=AF.Copy)

                yt = yt_p.tile([128, 9, BR, WO], f16, tag="yt")
                # all 6 plane-0/1 taps in ONE op and the 3 shifted singles in a
                # second: hand-built APs decompose the x row index as i + 2r
                xb, gcb, ytb = xsk[blk][:], gc[:], yt[:]
                XP, GP = CR * 2 * JW, 9 * BR * WO

                def xv(i0, ni, pl0, np_, co=0):
                    return bass.AP(xb.tensor, xb.offset + 2 * JW * i0 + JW * pl0 + co,
                                   ([[XP, 128], [2 * JW, ni]] if ni > 1 else [[XP, 128]])
                                   + ([[JW, np_]] if np_ > 1 else [])
                                   + [[4 * JW, BR], [1, WO]])

                def gv(t, tap0, ni, np_):
                    return bass.AP(t.tensor, t.offset + NPOS * tap0,
                                   ([[GP, 128], [3 * NPOS, ni]] if ni > 1 else [[GP, 128]])
                                   + ([[NPOS, np_]] if np_ > 1 else [])
                                   + [[WO, BR], [1, WO]])

                # taps {0,1,3,4}: PE+ACT path, available first
                nc.vector.tensor_tensor(out=gv(ytb, 0, 2, 2), in0=xv(0, 2, 0, 2),
                                        in1=gv(gcb, 0, 2, 2), op=MULT)
                # singles {2,5}: plane-0 shifted, ACT path
                nc.vector.tensor_tensor(out=gv(ytb, 2, 2, 1), in0=xv(0, 2, 0, 1, co=1),
                                        in1=gv(gcb, 2, 2, 1), op=MULT)
                # taps {6,7} + single {8}: DMA-broadcast path, available last
                nc.vector.tensor_tensor(out=gv(ytb, 6, 1, 2), in0=xv(2, 1, 0, 2),
                                        in1=gv(gcb, 6, 1, 2), op=MULT)
                nc.vector.tensor_tensor(out=gv(ytb, 8, 1, 1), in0=xv(2, 1, 0, 1, co=1),
                                        in1=gv(gcb, 8, 1, 1), op=MULT)

                t4 = tr_p.tile([128, 4, BR, WO], f16, tag="t4")
                nc.vector.tensor_tensor(out=t4[:], in0=yt[:, 0:8:2], in1=yt[:, 1:8:2], op=ADD)
                t2 = tr_p.tile([128, 2, BR, WO], f16, tag="t2")
                nc.vector.tensor_tensor(out=t2[:], in0=t4[:, 0:4:2], in1=t4[:, 1:4:2], op=ADD)
                tA = tr_p.tile([128, BR, WO], f16, tag="tA")
                nc.vector.tensor_tensor(out=tA[:], in0=t2[:, 0], in1=t2[:, 1], op=ADD)
                y = tr_p.tile([128, BR, WO], f16, tag="y")
                nc.vector.tensor_tensor(out=y[:], in0=tA[:], in1=yt[:, 8], op=ADD)
                nc.sync.dma_start(out=out[:, BR * blk:BR * (blk + 1), :], in_=y[:])

            with tc.tile_wait_until(0.0045):
                gbnA = g_emit(0)
            with tc.tile_wait_until(0.005):
                unfold_emit(0, gbnA)
            with tc.tile_wait_until(0.0055):
                gbnB = g_emit(1)
            with tc.tile_wait_until(0.006):
                unfold_emit(1, gbnA)
            with tc.tile_wait_until(0.007):
                unfold_emit(2, gbnB)
            with tc.tile_wait_until(0.008):
                unfold_emit(3, gbnB)

    if not for_sim and not nc.is_finalized():
        nc.finalize()
    return nc


def _prep_inputs(x, conv_w, bn_gamma, bn_beta, bn_mean, bn_var):
    cst, wt, ohs, ones = _build_consts(conv_w, bn_gamma, bn_beta, bn_mean, bn_var)
    xp = np.pad(np.asarray(x, np.float32), ((0, 0), (0, 0), (1, 1), (1, 1)),
                mode="reflect").astype(np.float16)                    # [8,64,130,130]
    in_maps = []
    for n in range(N):
        xc = np.concatenate([xp[n, :, 0:RS, :], xp[n, :, 64:64 + RS, :]], axis=0)
        xpl = np.zeros((128, RS, 2, JW), np.float16)
        xpl[:, :, 0, 0:65] = xc[:, :, 0:130:2]
        xpl[:, :, 1, 0:65] = xc[:, :, 1:130:2]
        im = {"xin": xpl, "cin": cst, "win": wt}
        for g in range(len(GRP)):
            im[f"gin{g}"] = ohs[g]
            im[f"oin{g}"] = ones[g]
        in_maps.append(im)
    return in_maps


def _gather(results):
    out = np.empty((N, C, HO, WO), np.float32)
    for n in range(N):
        d = results[n]["out"].astype(np.float32)
        out[n, :, 0:HOC, :] = d[0:64]
        out[n, :, HOC:, :] = d[64:128]
    return out


def _enable_axon_trace():
    if _STATE.get("trace_hooked"):
        return
    import types
    import antenv
    from concourse import bass_utils
    mod = types.ModuleType("antenv.axon_hooks")
    mod._hook = None
    mod.set_axon_ntff_profile_hook = lambda h: setattr(mod, "_hook", h)
    mod.get_axon_ntff_profile_hook = lambda: mod._hook
    sys.modules["antenv.axon_hooks"] = mod
    antenv.axon_hooks = mod
    from trn_agent_boot.trn_boot import _ntff_profile_via_ctypes
    mod._hook = _ntff_profile_via_ctypes("/opt/axon/libaxon_pjrt.so")
    bass_utils.upload_artifacts = lambda tmpdir: tmpdir
    _STATE["trace_hooked"] = True


def run(x, conv_w, bn_gamma, bn_beta, bn_mean, bn_var, trace=False):
    from concourse.bass_utils import run_bass_kernel_spmd
    if trace:
        _enable_axon_trace()
    if "nc" not in _STATE:
        _STATE["nc"] = _build_bass()
    in_maps = _prep_inputs(x, conv_w, bn_gamma, bn_beta, bn_mean, bn_var)
    res = run_bass_kernel_spmd(_STATE["nc"], in_maps, list(range(N)), trace=trace)
    _STATE["last"] = res
    return _gather(res.results)


def kernel(x, conv_w, bn_gamma, bn_beta, bn_mean, bn_var):
    return run(x, conv_w, bn_gamma, bn_beta, bn_mean, bn_var,
               trace=bool(int(os.environ.get("KERNEL_TRACE", "0"))))
